# revision 5
# baseline (speedup 1.0000x reference)
"""Trainium2 Bass kernel for nn_ChannelSELayerOwn (topk channel masking).

Reference computation (per batch sample b of 8, data-parallel across 8 cores):
  y   = mean(x[b], axis=(D,H,W))                       # (64,)
  h   = leaky_relu(w1 @ y + b1, 0.01)                  # (64,)
  z   = w2 @ h + b2                                    # (64,) pre-sigmoid logits
  idx = top_8 indices of sigmoid(z) == top_8 of z      # sigmoid is monotonic
  out[b] = x[b, idx]                                   # (8, D, H, W), bit-exact copy

Device kernel per core (one sample):
  phase A: stream x (viewed as 128 x 55296) through SBUF on THREE DMA
           queues at once (sync HWDGE ring ~56%, scalar HWDGE ring ~38%,
           gpsimd SWDGE ~6%) -- each queue has its own tile pool so a slow
           queue can never stall the others; each tile's column-sum is
           split between the DVE (reduce_sum) and the Act engine
           (activation Copy with accum_out) sized to their clock rates;
           the last tiles are small so the final reduce tail is ~1us
  phase B: one FC1 matmul over the two engines' partial-sum totals (the
           pair-reduce and 1/DHW divisor are folded into the host-prepared
           W1R weight), leaky on DVE, FC2 in row layout, top-8 via the DVE
           max8/max-index instructions on the pre-sigmoid logits (sigmoid
           is monotonic); all matmuls fp32 (top-8/9 logit gaps ~1e-4);
           winner indices are pre-multiplied into element offsets on DVE
           so the phase-C dispatch needs only a shift, not a multiply
  phase C: the 8 winning channels are copied HBM->HBM with plain DMAs
           using runtime (register) source offsets, spread over all three
           queues -- no SBUF staging, no SWDGE indirect gather
"""

import os

import numpy as np

import concourse.bacc as bacc
import concourse.bass as bass
import concourse.mybir as mybir
from concourse import tile
from concourse.bass_utils import run_bass_kernel_spmd

F32 = mybir.dt.float32
U32 = mybir.dt.uint32

B, C, D, H, W = 8, 64, 48, 48, 48
M = D * H * W              # 110592 elements per channel
R_TOP = 8                  # channels kept
NEG_SLOPE = 0.01
N_CORES = 8

TF = 6912                  # streaming tile free-dim (55296 = 8 * 6912)
NT = (M * C // 128) // TF  # 8 full-size streaming tiles
TAIL_SPLIT = 4             # the last streaming tile is split this many ways
TFS = TF // TAIL_SPLIT     # 1728-wide sub-tiles for a short reduce tail

# DVE runs at 0.96 GHz, Act at 1.2 GHz (both 1 elem/cycle/partition):
# split each tile's columns so both finish together (Act pays a larger
# fixed SBUF-access cost, so small tiles tilt further toward DVE)
DVE_TF = 3200              # DVE columns per big tile
DVE_TFS = 960              # DVE columns per tail sub-tile

# results of the most recent run_bass_kernel_spmd call (for test harness use)
LAST_RESULTS = None
_NC_CACHE = None


def build_nc():
    nc = bacc.Bacc("TRN2", target_bir_lowering=False)

    x_d = nc.dram_tensor("x", [C, M], F32, kind="ExternalInput")
    w1r_d = nc.dram_tensor("w1r", [128, C], F32, kind="ExternalInput")
    b1c_d = nc.dram_tensor("b1c", [C, 1], F32, kind="ExternalInput")
    w2t_d = nc.dram_tensor("w2t", [C, C], F32, kind="ExternalInput")
    b2r_d = nc.dram_tensor("b2r", [1, C], F32, kind="ExternalInput")
    out_d = nc.dram_tensor("out", [R_TOP, M], F32, kind="ExternalOutput")

    # x as 128 partitions x 55296: partition 2c+t holds half t of channel c
    x_stream = x_d[:].rearrange("c (t m) -> (c t) m", t=2)
    # flat view for the phase-C dynamic-offset channel copies
    x_flat = x_d[:].rearrange("c m -> (c m)")

    # streamed units: 7 big tiles b0..b6 + 4 tail sub-tiles s0..s3,
    # split over the three queues; emitted in expected completion order
    # (healthy-queue estimate) so the in-order engines rarely stall
    units = []  # (queue, kind, index)
    for j in range(NT - 1):
        q = "sync" if j % 2 == 0 else "scalar"
        units.append((q, "big", j))
    units.append(("pool", "small", 0))
    units.append(("pool", "small", 1))
    units.append(("scalar", "small", 2))
    units.append(("sync", "small", 3))
    # expected-completion emission order: pool smalls land first (tiny,
    # early), then big tiles alternate sync/scalar, then trailing smalls
    emit_order = [7, 8, 0, 1, 2, 3, 4, 5, 6, 9, 10]
    NUNIT = len(units)

    with tile.TileContext(nc) as tc:
        with (
            tc.tile_pool(name="consts", bufs=1) as cpool,
            tc.tile_pool(name="stream1", bufs=2) as spool1,
            tc.tile_pool(name="stream2", bufs=2) as spool2,
            tc.tile_pool(name="stream3", bufs=2) as spool3,
            tc.tile_pool(name="small", bufs=1) as mpool,
            tc.tile_pool(name="psum", bufs=1, space="PSUM") as ppool,
        ):
            w1r = cpool.tile([128, C], F32)
            nc.scalar.dma_start(w1r[:], w1r_d[:])
            w2t = cpool.tile([C, C], F32)
            nc.scalar.dma_start(w2t[:], w2t_d[:])
            b1c = cpool.tile([C, 1], F32)
            nc.scalar.dma_start(b1c[:], b1c_d[:])
            b2r = cpool.tile([1, C], F32)
            nc.scalar.dma_start(b2r[:], b2r_d[:])

            # ---- phase A: streaming channel sums ----
            ctxA = nc.named_scope("phaseA"); ctxA.__enter__()
            # separate partial tiles per engine so the two engines never
            # touch the same tile (no cross-engine WAW hazards)
            partials_v = mpool.tile([128, NUNIT], F32)
            partials_a = mpool.tile([128, NUNIT], F32)
            # Act's activation needs a full-size main output; it is garbage
            # and reused every iteration (serializes Act with itself only)
            adump = mpool.tile([128, TF - DVE_TF], F32)

            def unit_ap(kind, index):
                if kind == "big":
                    lo = index * TF
                    return x_stream[:, lo : lo + TF], TF, DVE_TF
                lo = (NT - 1) * TF + index * TFS
                return x_stream[:, lo : lo + TFS], TFS, DVE_TFS

            engines = {"sync": nc.sync, "scalar": nc.scalar, "pool": nc.gpsimd}
            pools = {"sync": spool1, "scalar": spool2, "pool": spool3}

            # issue every DMA first (per-queue FIFO program order: sync and
            # scalar interleave big tiles, pool takes two early smalls)
            tiles = {}
            for u in range(NUNIT):
                q, kind, index = units[u]
                ap, cols, _ = unit_ap(kind, index)
                t = pools[q].tile([128, cols], F32, tag=f"{q}_{kind}")
                engines[q].dma_start(t[:], ap)
                tiles[u] = t

            # reduces in expected-completion order; each unit owns one
            # column in each engine's partials tile
            for c, u in enumerate(emit_order):
                q, kind, index = units[u]
                _, cols, dcols = unit_ap(kind, index)
                t = tiles[u]
                nc.vector.reduce_sum(
                    partials_v[:, c : c + 1], t[:, :dcols],
                    axis=mybir.AxisListType.X,
                )
                nc.scalar.activation(
                    adump[:, : cols - dcols], t[:, dcols:cols],
                    mybir.ActivationFunctionType.Copy,
                    accum_out=partials_a[:, c : c + 1],
                )

            ctxA.__exit__(None, None, None)
            # ---- phase B: totals -> FC1 -> leaky -> FC2 -> top-8 ----
            ctxB = nc.named_scope("phaseB"); ctxB.__enter__()
            tot_v = mpool.tile([128, 1], F32)
            nc.vector.reduce_sum(tot_v[:], partials_v[:], axis=mybir.AxisListType.X)
            tot_a = mpool.tile([128, 1], F32)
            nc.vector.reduce_sum(tot_a[:], partials_a[:], axis=mybir.AxisListType.X)

            # h_pre = W1R.T @ (tot_v + tot_a), accumulated in PSUM; the
            # pair-reduce and 1/M divisor are folded into w1r host-side
            h_ps = ppool.tile([C, 1], F32)
            nc.tensor.matmul(h_ps[:], lhsT=w1r[:], rhs=tot_v[:], start=True, stop=False)
            nc.tensor.matmul(h_ps[:], lhsT=w1r[:], rhs=tot_a[:], start=False, stop=True)

            # h = leaky_relu(h_pre + b1) on DVE (single-engine phase B
            # avoids Act-table loads and extra cross-engine wakeups)
            h = mpool.tile([C, 1], F32)
            nc.vector.tensor_add(h[:], h_ps[:], b1c[:])
            h_scaled = mpool.tile([C, 1], F32)
            nc.vector.tensor_scalar_mul(h_scaled[:], h[:], NEG_SLOPE)
            h_act = mpool.tile([C, 1], F32)
            nc.vector.tensor_tensor(h_act[:], h[:], h_scaled[:], op=mybir.AluOpType.max)

            # z in row layout directly: z_row = h.T @ w2.T  (lhsT=h, rhs=w2t)
            zrow_ps = ppool.tile([1, C], F32)
            nc.tensor.matmul(zrow_ps[:], lhsT=h_act[:], rhs=w2t[:], start=True, stop=True)
            zrow = mpool.tile([1, C], F32)
            nc.vector.tensor_add(zrow[:], zrow_ps[:], b2r[:])

            m8 = mpool.tile([1, R_TOP], F32)
            nc.vector.max(m8[:], zrow[:])
            idx8 = mpool.tile([1, R_TOP], U32)
            nc.vector.max_index(idx8[:], m8[:], zrow[:])

            # pre-multiply winners into element offsets (idx * M) in fp32
            # (exact: 63*110592 < 2^24), so the phase-C AP lowering is a
            # shift instead of a register multiply chain
            idx8f = mpool.tile([1, R_TOP], F32)
            nc.vector.tensor_copy(idx8f[:], idx8[:])
            off8f = mpool.tile([1, R_TOP], F32)
            nc.vector.tensor_scalar_mul(off8f[:], idx8f[:], float(M))
            off8 = mpool.tile([1, R_TOP], U32)
            nc.vector.tensor_copy(off8[:], off8f[:])

            ctxB.__exit__(None, None, None)
            # ---- phase C: copy the selected channels HBM->HBM ----
            ctxC = nc.named_scope("phaseC"); ctxC.__enter__()
            _, off_vals = nc.values_load_multi_w_load_instructions(
                off8[:1, :],
                engines=[
                    mybir.EngineType.SP,
                    mybir.EngineType.Activation,
                    mybir.EngineType.Pool,
                ],
                min_val=0,
                max_val=(C - 1) * M,
                skip_runtime_bounds_check=True,
            )
            copy_eng = [
                nc.sync, nc.scalar, nc.gpsimd,
                nc.sync, nc.scalar, nc.gpsimd,
                nc.sync, nc.scalar,
            ]
            for r in range(R_TOP):
                copy_eng[r].dma_start(
                    out_d[r : r + 1, :], x_flat[bass.ds(off_vals[r], M)]
                )

            ctxC.__exit__(None, None, None)

    nc.compile()
    return nc


def _aux_inputs(w1, b1, w2, b2):
    # R[p, p//2] = 1/M so that R.T @ partition_sums = per-channel means
    rmat = np.zeros((128, C), dtype=np.float32)
    rmat[np.arange(128), np.arange(128) // 2] = np.float32(1.0 / M)
    return {
        "w1r": np.ascontiguousarray(rmat @ w1.T, dtype=np.float32),
        "b1c": np.ascontiguousarray(b1.reshape(C, 1), dtype=np.float32),
        "w2t": np.ascontiguousarray(w2.T, dtype=np.float32),
        "b2r": np.ascontiguousarray(b2.reshape(1, C), dtype=np.float32),
    }


def kernel(x, w1, b1, w2, b2):
    global LAST_RESULTS
    x = np.asarray(x, dtype=np.float32)
    aux = _aux_inputs(
        np.asarray(w1, np.float32), np.asarray(b1, np.float32),
        np.asarray(w2, np.float32), np.asarray(b2, np.float32),
    )
    global _NC_CACHE
    if _NC_CACHE is None:
        _NC_CACHE = build_nc()
    nc = _NC_CACHE
    in_maps = [
        {"x": np.ascontiguousarray(x[b].reshape(C, M)), **aux} for b in range(B)
    ]
    # the axon-tunneled device occasionally throws transient INTERNAL errors
    # (e.g. after an earlier aborted run wedged it); retry a couple of times
    res = None
    for attempt in range(3):
        try:
            res = run_bass_kernel_spmd(
                nc,
                in_maps,
                core_ids=list(range(N_CORES)),
                trace=bool(int(os.environ.get("BASS_PROFILE", "0"))),
            )
            break
        except Exception:
            if attempt == 2:
                raise
    LAST_RESULTS = res
    out = np.stack([res.results[b]["out"] for b in range(B)], axis=0)
    return out.reshape(B, R_TOP, D, H, W)


# revision 6
# speedup vs baseline: 1.2496x; 1.2496x over previous
"""Trainium2 Bass kernel for nn_ChannelSELayerOwn (topk channel masking).

Reference computation (per batch sample b of 8, data-parallel across 8 cores):
  y   = mean(x[b], axis=(D,H,W))                       # (64,)
  h   = leaky_relu(w1 @ y + b1, 0.01)                  # (64,)
  z   = w2 @ h + b2                                    # (64,) pre-sigmoid logits
  idx = top_8 indices of sigmoid(z) == top_8 of z      # sigmoid is monotonic
  out[b] = x[b, idx]                                   # (8, D, H, W), bit-exact copy

Device kernel per core (one sample):
  phase A: stream x (viewed as 128 x 55296) through SBUF on the sync-engine
           HWDGE ring in program order -- a single ring saturates the
           ~440 GB/s aggregate HBM read cap (measured: a second ring or the
           SWDGE queue only steals from the same cap); each tile's
           column-sum is split between the DVE (reduce_sum) and the Act
           engine (activation Copy with accum_out) sized to their clock
           rates so each runs at ~45% duty and never lags the stream; the
           last tile is split into 4 small sub-tiles so the final reduce
           adds ~1us after the last load lands
  phase B: one FC1 matmul over the two engines' partial-sum totals (the
           pair-reduce and 1/DHW divisor are folded into the host-prepared
           W1R weight), leaky on DVE, FC2 in row layout, top-8 via the DVE
           max8/max-index instructions on the pre-sigmoid logits (sigmoid
           is monotonic); all matmuls fp32 (top-8/9 logit gaps ~1e-4);
           winner indices are pre-multiplied into element offsets on DVE
           so the phase-C dispatch needs only a shift, not a multiply
  phase C: the 8 winning channels are copied HBM->HBM with plain HWDGE
           DMAs using runtime (register) source offsets, alternating
           between the sync and scalar rings -- no SBUF staging, no SWDGE
           indirect gather (packet-rate-limited to ~180 GB/s)
"""

import os

import numpy as np

import concourse.bacc as bacc
import concourse.bass as bass
import concourse.mybir as mybir
from concourse import tile
from concourse.bass_utils import run_bass_kernel_spmd

F32 = mybir.dt.float32
U32 = mybir.dt.uint32

B, C, D, H, W = 8, 64, 48, 48, 48
M = D * H * W              # 110592 elements per channel
R_TOP = 8                  # channels kept
NEG_SLOPE = 0.01
N_CORES = 8

TF = 6912                  # streaming tile free-dim (55296 = 8 * 6912)
NT = (M * C // 128) // TF  # 8 full-size streaming tiles
TAIL_SPLIT = 4             # the last streaming tile is split this many ways
TFS = TF // TAIL_SPLIT     # 1728-wide sub-tiles for a short reduce tail

# DVE runs at 0.96 GHz, Act at 1.2 GHz (both 1 elem/cycle/partition):
# split each tile's columns so both finish together (Act pays a larger
# fixed SBUF-access cost, so small tiles tilt further toward DVE)
DVE_TF = 3200              # DVE columns per big tile
DVE_TFS = 960              # DVE columns per tail sub-tile

# results of the most recent run_bass_kernel_spmd call (for test harness use)
LAST_RESULTS = None
_NC_CACHE = None


def build_nc():
    nc = bacc.Bacc("TRN2", target_bir_lowering=False)

    x_d = nc.dram_tensor("x", [C, M], F32, kind="ExternalInput")
    w1r_d = nc.dram_tensor("w1r", [128, C], F32, kind="ExternalInput")
    b1c_d = nc.dram_tensor("b1c", [C, 1], F32, kind="ExternalInput")
    w2t_d = nc.dram_tensor("w2t", [C, C], F32, kind="ExternalInput")
    b2r_d = nc.dram_tensor("b2r", [1, C], F32, kind="ExternalInput")
    out_d = nc.dram_tensor("out", [R_TOP, M], F32, kind="ExternalOutput")

    # x as 128 partitions x 55296: partition 2c+t holds half t of channel c
    x_stream = x_d[:].rearrange("c (t m) -> (c t) m", t=2)
    # flat view for the phase-C dynamic-offset channel copies
    x_flat = x_d[:].rearrange("c m -> (c m)")

    NUNIT = (NT - 1) + TAIL_SPLIT   # 11 streamed units

    with tile.TileContext(nc) as tc:
        with (
            tc.tile_pool(name="consts", bufs=1) as cpool,
            tc.tile_pool(name="stream", bufs=5) as spool,
            tc.tile_pool(name="small", bufs=1) as mpool,
            tc.tile_pool(name="psum", bufs=1, space="PSUM") as ppool,
        ):
            w1r = cpool.tile([128, C], F32)
            nc.scalar.dma_start(w1r[:], w1r_d[:])
            w2t = cpool.tile([C, C], F32)
            nc.scalar.dma_start(w2t[:], w2t_d[:])
            b1c = cpool.tile([C, 1], F32)
            nc.scalar.dma_start(b1c[:], b1c_d[:])
            b2r = cpool.tile([1, C], F32)
            nc.scalar.dma_start(b2r[:], b2r_d[:])

            # ---- phase A: streaming channel sums ----
            ctxA = nc.named_scope("phaseA"); ctxA.__enter__()
            # separate partial tiles per engine so the two engines never
            # touch the same tile (no cross-engine WAW hazards)
            partials_v = mpool.tile([128, NUNIT], F32)
            partials_a = mpool.tile([128, NUNIT], F32)
            # Act's activation needs a full-size main output; it is garbage
            # and reused every iteration (serializes Act with itself only)
            adump = mpool.tile([128, TF - DVE_TF], F32)

            def unit_reduce(xt, cols, dcols, c):
                nc.vector.reduce_sum(
                    partials_v[:, c : c + 1], xt[:, :dcols],
                    axis=mybir.AxisListType.X,
                )
                nc.scalar.activation(
                    adump[:, : cols - dcols], xt[:, dcols:cols],
                    mybir.ActivationFunctionType.Copy,
                    accum_out=partials_a[:, c : c + 1],
                )

            col = 0
            for j in range(NT - 1):
                xt = spool.tile([128, TF], F32, tag="xt")
                nc.sync.dma_start(xt[:], x_stream[:, j * TF : (j + 1) * TF])
                unit_reduce(xt, TF, DVE_TF, col)
                col += 1
            base = (NT - 1) * TF
            for j in range(TAIL_SPLIT):
                xts = spool.tile([128, TFS], F32, tag="xts")
                nc.sync.dma_start(
                    xts[:], x_stream[:, base + j * TFS : base + (j + 1) * TFS]
                )
                unit_reduce(xts, TFS, DVE_TFS, col)
                col += 1

            ctxA.__exit__(None, None, None)
            # ---- phase B: totals -> FC1 -> leaky -> FC2 -> top-8 ----
            ctxB = nc.named_scope("phaseB"); ctxB.__enter__()
            tot_v = mpool.tile([128, 1], F32)
            nc.vector.reduce_sum(tot_v[:], partials_v[:], axis=mybir.AxisListType.X)
            tot_a = mpool.tile([128, 1], F32)
            nc.vector.reduce_sum(tot_a[:], partials_a[:], axis=mybir.AxisListType.X)

            # h_pre = W1R.T @ (tot_v + tot_a), accumulated in PSUM; the
            # pair-reduce and 1/M divisor are folded into w1r host-side
            h_ps = ppool.tile([C, 1], F32)
            nc.tensor.matmul(h_ps[:], lhsT=w1r[:], rhs=tot_v[:], start=True, stop=False)
            nc.tensor.matmul(h_ps[:], lhsT=w1r[:], rhs=tot_a[:], start=False, stop=True)

            # h = leaky_relu(h_pre + b1) on DVE (single-engine phase B
            # avoids Act table loads and extra cross-engine wakeups)
            h = mpool.tile([C, 1], F32)
            nc.vector.tensor_add(h[:], h_ps[:], b1c[:])
            h_scaled = mpool.tile([C, 1], F32)
            nc.vector.tensor_scalar_mul(h_scaled[:], h[:], NEG_SLOPE)
            h_act = mpool.tile([C, 1], F32)
            nc.vector.tensor_tensor(h_act[:], h[:], h_scaled[:], op=mybir.AluOpType.max)

            # z in row layout directly: z_row = h.T @ w2.T  (lhsT=h, rhs=w2t)
            zrow_ps = ppool.tile([1, C], F32)
            nc.tensor.matmul(zrow_ps[:], lhsT=h_act[:], rhs=w2t[:], start=True, stop=True)
            zrow = mpool.tile([1, C], F32)
            nc.vector.tensor_add(zrow[:], zrow_ps[:], b2r[:])

            m8 = mpool.tile([1, R_TOP], F32)
            nc.vector.max(m8[:], zrow[:])
            idx8 = mpool.tile([1, R_TOP], U32)
            nc.vector.max_index(idx8[:], m8[:], zrow[:])

            # pre-multiply winners into element offsets (idx * M) in fp32
            # (exact: 63*110592 < 2^24), so the phase-C AP lowering is a
            # shift instead of a register multiply chain
            idx8f = mpool.tile([1, R_TOP], F32)
            nc.vector.tensor_copy(idx8f[:], idx8[:])
            off8f = mpool.tile([1, R_TOP], F32)
            nc.vector.tensor_scalar_mul(off8f[:], idx8f[:], float(M))
            off8 = mpool.tile([1, R_TOP], U32)
            nc.vector.tensor_copy(off8[:], off8f[:])

            ctxB.__exit__(None, None, None)
            # ---- phase C: copy the selected channels HBM->HBM ----
            ctxC = nc.named_scope("phaseC"); ctxC.__enter__()
            _, off_vals = nc.values_load_multi_w_load_instructions(
                off8[:1, :],
                engines=[mybir.EngineType.SP, mybir.EngineType.Activation],
                min_val=0,
                max_val=(C - 1) * M,
                skip_runtime_bounds_check=True,
            )
            for r in range(R_TOP):
                eng = nc.sync if r % 2 == 0 else nc.scalar
                eng.dma_start(
                    out_d[r : r + 1, :], x_flat[bass.ds(off_vals[r], M)]
                )

            ctxC.__exit__(None, None, None)

    nc.compile()
    return nc


def _aux_inputs(w1, b1, w2, b2):
    # R[p, p//2] = 1/M so that R.T @ partition_sums = per-channel means
    rmat = np.zeros((128, C), dtype=np.float32)
    rmat[np.arange(128), np.arange(128) // 2] = np.float32(1.0 / M)
    return {
        "w1r": np.ascontiguousarray(rmat @ w1.T, dtype=np.float32),
        "b1c": np.ascontiguousarray(b1.reshape(C, 1), dtype=np.float32),
        "w2t": np.ascontiguousarray(w2.T, dtype=np.float32),
        "b2r": np.ascontiguousarray(b2.reshape(1, C), dtype=np.float32),
    }


def kernel(x, w1, b1, w2, b2):
    global LAST_RESULTS
    x = np.asarray(x, dtype=np.float32)
    aux = _aux_inputs(
        np.asarray(w1, np.float32), np.asarray(b1, np.float32),
        np.asarray(w2, np.float32), np.asarray(b2, np.float32),
    )
    global _NC_CACHE
    if _NC_CACHE is None:
        _NC_CACHE = build_nc()
    nc = _NC_CACHE
    in_maps = [
        {"x": np.ascontiguousarray(x[b].reshape(C, M)), **aux} for b in range(B)
    ]
    # the axon-tunneled device occasionally throws transient INTERNAL errors
    # (e.g. after an earlier aborted run wedged it); retry a couple of times
    res = None
    for attempt in range(3):
        try:
            res = run_bass_kernel_spmd(
                nc,
                in_maps,
                core_ids=list(range(N_CORES)),
                trace=bool(int(os.environ.get("BASS_PROFILE", "0"))),
            )
            break
        except Exception:
            if attempt == 2:
                raise
    LAST_RESULTS = res
    out = np.stack([res.results[b]["out"] for b in range(B)], axis=0)
    return out.reshape(B, R_TOP, D, H, W)
